# revision 27
# baseline (speedup 1.0000x reference)
"""nn_CrossMamba Trainium2 kernel.

Bidirectional Mamba over x = concat(context+seg_c, query+seg_q) (T=4096).
Sharding: 8 cores = (direction 2) x (batch 2) x (d_inner half 2); no
collectives — each core computes a partial out-projection over its 512
channels; the host sums partials and un-flips the backward direction.

Per-core layout is [channel-partitions, time-free] throughout. The
selective scan runs on the DVE tensor_tensor_scan instruction, one
(state, d-tile, time-chunk) at a time:  h = exp(A_s*dt) * h + (dt*xc*B_s).
exp(A_s*dt) comes from ACT Exp with per-partition scale A[:, s].
B_s/C_s rows are broadcast across partitions with a K=1 PE matmul
(ones[1,128]^T @ row) directly into PSUM.
"""

import sys

_TRN_REPO = "/opt/trn_rl_repo"
if _TRN_REPO not in sys.path:
    sys.path.insert(0, _TRN_REPO)

import numpy as np

import concourse.bass as bass
import concourse.mybir as mybir
import concourse.tile as tile
from concourse import bacc
from concourse.bass import ds, ts

F32 = mybir.dt.float32
F32R = mybir.dt.float32r
BF16 = mybir.dt.bfloat16
AF = mybir.ActivationFunctionType
OP = mybir.AluOpType

T = 4096          # total time (Lc + Lq)
TC = 512          # time chunk
NCH = T // TC
DM = 512          # d_model
DF = 1024         # d_inner full
DH = 512          # d_inner half (per core)
S = 16            # d_state
R = 32            # dt_rank
KC = 4            # d_conv
NKM = DM // 128   # 4  K-tiles for in_proj (contraction over d_model)
NDF = DF // 128   # 8  d-tiles full
NDH = DH // 128   # 4  d-tiles half
NMO = DM // 128   # 4  M-tiles for out_proj


def build_program(stage="full"):
    """Build the SPMD per-core program. stage in
    {"inproj", "conv", "dbl", "dt", "scan", "full"} for staged debugging —
    each early stage writes its intermediate to outT-shaped DRAM (padded)."""
    nc = bacc.Bacc("TRN2", target_bir_lowering=False, debug=False, num_devices=8)

    xT = nc.dram_tensor("xT", [DM, T], F32R, kind="ExternalInput")
    Win_l = nc.dram_tensor("Win_l", [DM, DF + DH], F32R, kind="ExternalInput")
    convw = nc.dram_tensor("convw", [DF, KC], F32, kind="ExternalInput")
    convb = nc.dram_tensor("convb", [DF, 1], F32, kind="ExternalInput")
    Wx_l = nc.dram_tensor("Wx_l", [DF, R + 2 * S], F32R, kind="ExternalInput")
    Wdt_l = nc.dram_tensor("Wdt_l", [R + 1, DH], F32R, kind="ExternalInput")
    A_h = nc.dram_tensor("A_h", [DH, S], F32, kind="ExternalInput")
    D_h = nc.dram_tensor("D_h", [DH, 1], F32, kind="ExternalInput")
    Wout_l = nc.dram_tensor("Wout_l", [DH, DM], F32R, kind="ExternalInput")
    sel = nc.dram_tensor("sel", [2 * S, 2 * S * 128], F32R, kind="ExternalInput")
    ones_d = nc.dram_tensor("ones_d", [1, T], F32R, kind="ExternalInput")

    if stage == "inproj":
        dbg = nc.dram_tensor("dbg", [DF + DH, T], F32R, kind="ExternalOutput")
    elif stage in ("conv", "dt"):
        dbg = nc.dram_tensor("dbg", [DF, T], F32R, kind="ExternalOutput")
    elif stage == "dbl":
        dbg = nc.dram_tensor("dbg", [R + 2 * S + 1, T], F32R, kind="ExternalOutput")
    elif stage == "scan":
        dbg = nc.dram_tensor("dbg", [DH, T], F32R, kind="ExternalOutput")
    else:
        outT = nc.dram_tensor("outT", [DM, T], F32R, kind="ExternalOutput")

    with tile.TileContext(nc) as tc:
        _emit(nc, tc, stage, locals())
    nc.compile()
    return nc


def _emit(nc, tc, stage, tens):
    xT, Win_l, convw, convb = tens["xT"], tens["Win_l"], tens["convw"], tens["convb"]
    Wx_l, Wdt_l, A_h, D_h, Wout_l = (
        tens["Wx_l"], tens["Wdt_l"], tens["A_h"], tens["D_h"], tens["Wout_l"])
    sel_d = tens["sel"]
    ones_d = tens["ones_d"]
    dbg = tens.get("dbg")
    outT = tens.get("outT")

    from contextlib import ExitStack
    ctx = ExitStack()
    with ctx:
        wpool = ctx.enter_context(tc.tile_pool(name="weights", bufs=1))
        xpool = ctx.enter_context(tc.tile_pool(name="xT", bufs=1))
        xipool = ctx.enter_context(tc.tile_pool(name="xi", bufs=1))
        xcpool = ctx.enter_context(tc.tile_pool(name="xc", bufs=1))
        xchpool = ctx.enter_context(tc.tile_pool(name="xch", bufs=1))
        zpool = ctx.enter_context(tc.tile_pool(name="z", bufs=1))
        dwpool = ctx.enter_context(tc.tile_pool(name="dtw", bufs=1))
        sipool = ctx.enter_context(tc.tile_pool(name="scanin", bufs=1))
        spool = ctx.enter_context(tc.tile_pool(name="scan", bufs=2))
        ypool = ctx.enter_context(tc.tile_pool(name="yacc", bufs=1))
        hpool = ctx.enter_context(tc.tile_pool(name="hstate", bufs=1))
        ps_mm = ctx.enter_context(tc.tile_pool(name="psmm", bufs=2, space="PSUM"))
        ps_bc = ctx.enter_context(tc.tile_pool(name="psbc", bufs=3, space="PSUM"))

        # --- persistent weights in SBUF ---
        w_in = []
        for k in range(NKM):
            t_ = wpool.tile([128, DF + DH], F32R, tag=f"win{k}", name=f"win{k}")
            nc.sync.dma_start(t_[:, :], Win_l[ts(k, 128), :])
            w_in.append(t_)
        w_x = []
        for k in range(NDF):
            t_ = wpool.tile([128, R + 2 * S], F32R, tag=f"wx{k}", name=f"wx{k}")
            nc.sync.dma_start(t_[:, :], Wx_l[ts(k, 128), :])
            w_x.append(t_)
        w_dt = wpool.tile([R + 1, DH], F32R, tag="wdt", name="wdt")
        nc.sync.dma_start(w_dt[:, :], Wdt_l[:, :])
        w_out = []
        for k in range(NDH):
            t_ = wpool.tile([128, DM], F32R, tag=f"wout{k}", name=f"wout{k}")
            nc.sync.dma_start(t_[:, :], Wout_l[ts(k, 128), :])
            w_out.append(t_)
        cw = []
        cb = []
        for k in range(NDF):
            t_ = wpool.tile([128, KC], F32, tag=f"cw{k}", name=f"cw{k}")
            nc.sync.dma_start(t_[:, :], convw[ts(k, 128), :])
            cw.append(t_)
            t_ = wpool.tile([128, 1], F32, tag=f"cb{k}", name=f"cb{k}")
            nc.sync.dma_start(t_[:, :], convb[ts(k, 128), :])
            cb.append(t_)
        a_sb = []
        d_sb = []
        for k in range(NDH):
            t_ = wpool.tile([128, S], F32, tag=f"a{k}", name=f"a{k}")
            nc.sync.dma_start(t_[:, :], A_h[ts(k, 128), :])
            a_sb.append(t_)
            t_ = wpool.tile([128, 1], F32, tag=f"dd{k}", name=f"dd{k}")
            nc.sync.dma_start(t_[:, :], D_h[ts(k, 128), :])
            d_sb.append(t_)
        halo = [wpool.tile([128, KC - 1], F32, tag=f"halo{k}", name=f"halo{k}") for k in range(NDF)]
        # persistent scan state [128, S] per half d-tile
        hstate = [hpool.tile([128, S], F32, tag=f"hs{k}", name=f"hs{k}") for k in range(NDH)]

        # half selection is encoded in host-side weight slicing:
        # Win_l z-columns, Wdt_l, A_h, D_h, Wout_l are all pre-sliced for this
        # core's half; xc half tiles are d-tiles [hoff, hoff+4) of the full 8.
        # The host picks hoff by passing Wdt/A/D/Wout for that half, but the
        # SCAN must read the matching xc tiles. To keep the program identical
        # across cores, the host instead REORDERS Wx_l's rows so that this
        # core's half occupies d-tiles 0..3. See host prep: xc tile j of the
        # kernel corresponds to host channel block perm[j].

        NT = TC // 512  # matmul N-chunks per time chunk

        for c in range(NCH):
            # ---- load xT chunk ----
            xt = [xpool.tile([128, TC], F32R, tag=f"xt{k}", name=f"xt{k}") for k in range(NKM)]
            for k in range(NKM):
                nc.sync.dma_start(xt[k][:, :], xT[ts(k, 128), ds(c * TC, TC)])

            # ---- in_proj + conv + silu per d-tile; dbl accumulates in PSUM
            # inside the loop so other-half xc tiles stay transient.
            # scanin_dt rows 0:32 = dt-rank dbl, row 32 = ones (from DRAM);
            # bc_sb rows = B(16)+C(16). All matmul operands at base 0. ----
            scanin_dt = sipool.tile([R + 1, TC], F32R, tag="scanin", name="scanin")
            bc_sb = sipool.tile([2 * S, TC], F32R, tag="bcsb", name="bcsb")
            nc.sync.dma_start(scanin_dt[R:R + 1, :], ones_d[:, ds(c * TC, TC)])
            psd2 = [ps_mm.tile([R, 512], F32, tag="dblps", name="dblps", bufs=2)
                    for _ in range(NT)]
            psb2 = [ps_mm.tile([2 * S, 512], F32, tag="dblps", name="dblps", bufs=2)
                    for _ in range(NT)]
            xc_t = []
            for j in range(NDF):
                xi = xipool.tile([128, KC - 1 + TC], F32, tag="xi", name="xi")
                for n in range(NT):
                    psn = ps_mm.tile([128, 512], F32, tag="mm", name="mm")
                    for k in range(NKM):
                        nc.tensor.matmul(
                            psn[:, :],
                            w_in[k][:, ds(j * 128, 128)],
                            xt[k][:, ds(n * 512, 512)],
                            start=(k == 0), stop=(k == NKM - 1),
                        )
                    nc.vector.tensor_copy(xi[:, ds(KC - 1 + n * 512, 512)], psn[:, :])
                # halo
                if c == 0:
                    nc.vector.memset(xi[:, 0:KC - 1], 0.0)
                else:
                    nc.vector.tensor_copy(xi[:, 0:KC - 1], halo[j][:, :])
                nc.vector.tensor_copy(halo[j][:, :], xi[:, ds(TC, KC - 1)])
                if stage == "inproj":
                    nc.gpsimd.dma_start(dbg[ts(j, 128), ds(c * TC, TC)],
                                        xi[:, ds(KC - 1, TC)])
                # conv: acc over taps
                xc = xcpool.tile([128, TC], F32R, tag="xc", name="xc") if j >= NDH else \
                    xchpool.tile([128, TC], F32R, tag=f"xch{j}", name=f"xch{j}")
                # first tap into xc, then 2 STT, final tap fused into silu bias?
                nc.vector.tensor_scalar_mul(xc[:, :], xi[:, 0:TC], cw[j][:, 0:1])
                for kk in range(1, KC):
                    nc.vector.scalar_tensor_tensor(
                        xc[:, :], xi[:, ds(kk, TC)], cw[j][:, kk:kk + 1], xc[:, :],
                        op0=OP.mult, op1=OP.add,
                    )
                # silu(xc + convb)
                nc.scalar.activation(xc[:, :], xc[:, :], AF.Silu, bias=cb[j][:, 0:1])
                xc_t.append(xc)
                if stage == "conv":
                    nc.sync.dma_start(dbg[ts(j, 128), ds(c * TC, TC)], xc[:, :])
                # dbl contributions of this d-tile (PSUM-accumulated over j)
                for n in range(NT):
                    nc.tensor.matmul(
                        psd2[n][:, :], w_x[j][:, 0:R],
                        xc[:, ds(n * 512, 512)],
                        start=(j == 0), stop=(j == NDF - 1),
                    )
                    nc.tensor.matmul(
                        psb2[n][:, :], w_x[j][:, R:R + 2 * S],
                        xc[:, ds(n * 512, 512)],
                        start=(j == 0), stop=(j == NDF - 1),
                    )

            # z tiles: M-tiles 8..11 of in_proj
            zg = []
            for j in range(NDH):
                z = zpool.tile([128, TC], F32R, tag=f"z{j}", name=f"z{j}")
                for n in range(NT):
                    psn = ps_mm.tile([128, 512], F32, tag="mm", name="mm")
                    for k in range(NKM):
                        nc.tensor.matmul(
                            psn[:, :],
                            w_in[k][:, ds(DF + j * 128, 128)],
                            xt[k][:, ds(n * 512, 512)],
                            start=(k == 0), stop=(k == NKM - 1),
                        )
                    # silu directly from psum
                    nc.scalar.activation(z[:, ds(n * 512, 512)], psn[:, :], AF.Silu)
                zg.append(z)

            # ---- collect dbl results from PSUM ----
            for n in range(NT):
                nc.vector.tensor_copy(scanin_dt[0:R, ds(n * 512, 512)], psd2[n][:, :])
                nc.vector.tensor_copy(bc_sb[:, ds(n * 512, 512)], psb2[n][:, :])
            if stage == "dbl":
                nc.sync.dma_start(dbg[0:R + 1, ds(c * TC, TC)], scanin_dt[:, :])
                nc.sync.dma_start(dbg[R + 1:R + 1 + 2 * S, ds(c * TC, TC)], bc_sb[:, :])

            # ---- dt + w  (softplus(p) = -ln(sigmoid(-p)); lns = ln(sigmoid(-p)) = -dt) ----
            lns_t, w_t = [], []
            for j in range(NDH):
                lns = dwpool.tile([128, TC], F32R, tag=f"lns{j}", name=f"lns{j}")
                w = dwpool.tile([128, TC], F32R, tag=f"w{j}", name=f"w{j}")
                for n in range(NT):
                    psn = ps_mm.tile([128, 512], F32, tag="mm", name="mm")
                    nc.tensor.matmul(
                        psn[:, :], w_dt[:, ds(j * 128, 128)],
                        scanin_dt[0:R + 1, ds(n * 512, 512)],
                        start=True, stop=True,
                    )
                    # s = sigmoid(-p) into lns slot, then ln in place
                    nc.scalar.activation(lns[:, ds(n * 512, 512)], psn[:, :],
                                         AF.Sigmoid, scale=-1.0)
                nc.scalar.activation(lns[:, :], lns[:, :], AF.Ln)
                # w = dt * xc_half = (-lns) * xc   (xc half = tiles 0..3, host reorder)
                nc.vector.scalar_tensor_tensor(w[:, :], lns[:, :], -1.0,
                                               xc_t[j][:, :], op0=OP.mult, op1=OP.mult)
                lns_t.append(lns)
                w_t.append(w)
                if stage == "dt":
                    nc.sync.dma_start(dbg[ts(j, 128), ds(c * TC, TC)], lns[:, :])

            if stage in ("inproj", "conv", "dbl", "dt"):
                continue

            # ---- scan over states ----
            yacc = [ypool.tile([128, TC], F32R, tag=f"ya{j}", name=f"ya{j}") for j in range(NDH)]
            for s in range(S):
                # broadcast B_s, C_s rows across 128 partitions via one-hot
                # selector matmul: out[:, t] = sel_s.T @ bc_sb (K=2S, base 0)
                selb = spool.tile([2 * S, 128], F32R, tag="selb", name="selb", bufs=2)
                nc.sync.dma_start(selb[:, :], sel_d[:, ts(s, 128)])
                selc = spool.tile([2 * S, 128], F32R, tag="selc", name="selc", bufs=2)
                nc.sync.dma_start(selc[:, :], sel_d[:, ts(S + s, 128)])
                bb, cc_ = [], []
                for n in range(NT):
                    pb = ps_bc.tile([128, 512], F32, tag="bc", name="bc")
                    nc.tensor.matmul(pb[:, :], selb[:, :],
                                     bc_sb[:, ds(n * 512, 512)],
                                     start=True, stop=True)
                    bb.append(pb)
                    pc = ps_bc.tile([128, 512], F32, tag="bc", name="bc")
                    nc.tensor.matmul(pc[:, :], selc[:, :],
                                     bc_sb[:, ds(n * 512, 512)],
                                     start=True, stop=True)
                    cc_.append(pc)
                for j in range(NDH):
                    dA = spool.tile([128, TC], F32R, tag="dA", name="dA")
                    nc.scalar.activation(dA[:, :], lns_t[j][:, :], AF.Exp,
                                         scale=a_sb[j][:, s:s + 1])
                    dBx = spool.tile([128, TC], F32R, tag="dBx", name="dBx")
                    for n in range(NT):
                        nc.vector.tensor_tensor(dBx[:, ds(n * 512, 512)],
                                                w_t[j][:, ds(n * 512, 512)],
                                                bb[n][:, :], op=OP.mult)
                    h = spool.tile([128, TC], F32R, tag="h", name="h")
                    init = 0.0 if c == 0 else hstate[j][:, s:s + 1]
                    nc.vector.tensor_tensor_scan(h[:, :], dA[:, :], dBx[:, :],
                                                 init, op0=OP.mult, op1=OP.add)
                    nc.vector.tensor_copy(hstate[j][:, s:s + 1], h[:, TC - 1:TC])
                    # yacc += h * C_s
                    if s == 0:
                        for n in range(NT):
                            nc.vector.tensor_tensor(yacc[j][:, ds(n * 512, 512)],
                                                    h[:, ds(n * 512, 512)],
                                                    cc_[n][:, :], op=OP.mult)
                    else:
                        ym = spool.tile([128, TC], F32R, tag="ym", name="ym", bufs=1)
                        for n in range(NT):
                            nc.vector.tensor_tensor(ym[:, ds(n * 512, 512)],
                                                    h[:, ds(n * 512, 512)],
                                                    cc_[n][:, :], op=OP.mult)
                        nc.vector.tensor_tensor(yacc[j][:, :], yacc[j][:, :],
                                                ym[:, :], op=OP.add)

            # ---- skip + gate + out_proj ----
            yg = []
            for j in range(NDH):
                y = ypool.tile([128, TC], F32R, tag=f"yg{j}", name=f"yg{j}")
                # y = yacc + xc*D
                nc.vector.scalar_tensor_tensor(y[:, :], xc_t[j][:, :],
                                               d_sb[j][:, 0:1], yacc[j][:, :],
                                               op0=OP.mult, op1=OP.add)
                if stage == "scan":
                    nc.sync.dma_start(dbg[ts(j, 128), ds(c * TC, TC)], y[:, :])
                    yg.append(y)
                    continue
                nc.vector.tensor_tensor(y[:, :], y[:, :], zg[j][:, :], op=OP.mult)
                yg.append(y)
            if stage == "scan":
                continue

            for m in range(NMO):
                for n in range(NT):
                    pso = ps_mm.tile([128, 512], F32, tag="mm", name="mm")
                    for k in range(NDH):
                        nc.tensor.matmul(
                            pso[:, :], w_out[k][:, ds(m * 128, 128)],
                            yg[k][:, ds(n * 512, 512)],
                            start=(k == 0), stop=(k == NDH - 1),
                        )
                    osb = ypool.tile([128, 512], F32R, tag="osb", name="osb", bufs=3)
                    nc.vector.tensor_copy(osb[:, :], pso[:, :])
                    nc.sync.dma_start(
                        outT[ts(m, 128), ds(c * TC + n * 512, 512)], osb[:, :])


# ---------------------------------------------------------------------------
# host side
# ---------------------------------------------------------------------------

_COMPILED = {}

# one-hot selector: sel[r, s*128 + m] = (r == s), for the B/C row broadcast
_SEL = np.zeros((2 * S, 2 * S * 128), np.float32)
for _s in range(2 * S):
    _SEL[_s, _s * 128:(_s + 1) * 128] = 1.0
_ONES = np.ones((1, T), np.float32)


class _CompiledSpmd:
    def __init__(self, nc, n_cores=8):
        import jax
        from jax.sharding import Mesh, PartitionSpec
        from jax.experimental.shard_map import shard_map
        from concourse.bass2jax import (
            _bass_exec_p, partition_id_tensor, install_neuronx_cc_hook)

        install_neuronx_cc_hook()
        self.jax = jax
        self.nc = nc
        self.n_cores = n_cores
        in_names, out_names, out_avals, zero_outs = [], [], [], []
        partition_name = nc.partition_id_tensor.name if nc.partition_id_tensor else None
        for alloc in nc.m.functions[0].allocations:
            if not isinstance(alloc, mybir.MemoryLocationSet):
                continue
            name = alloc.memorylocations[0].name
            if alloc.kind == "ExternalInput":
                if name != partition_name:
                    in_names.append(name)
            elif alloc.kind == "ExternalOutput":
                shape = tuple(alloc.tensor_shape)
                dtype = mybir.dt.np(alloc.dtype)
                out_avals.append(jax.core.ShapedArray(shape, dtype))
                out_names.append(name)
                zero_outs.append(np.zeros(shape, dtype))
        assert nc.dbg_addr is None
        self.in_names, self.out_names = in_names, out_names
        self.out_avals, self.zero_outs = out_avals, zero_outs
        all_in = list(in_names) + list(out_names)
        if partition_name is not None:
            all_in.append(partition_name)

        def _body(*args):
            operands = list(args)
            if partition_name is not None:
                operands.append(partition_id_tensor())
            return tuple(_bass_exec_p.bind(
                *operands,
                out_avals=tuple(out_avals), in_names=tuple(all_in),
                out_names=tuple(out_names),
                lowering_input_output_aliases=(),
                sim_require_finite=True, sim_require_nnan=True, nc=nc))

        devices = jax.devices()[:n_cores]
        mesh = Mesh(np.asarray(devices), ("core",))
        n_outs = len(out_avals)
        self.fn = jax.jit(
            shard_map(_body, mesh=mesh,
                      in_specs=(PartitionSpec("core"),) * (len(in_names) + n_outs),
                      out_specs=(PartitionSpec("core"),) * n_outs,
                      check_rep=False),
            keep_unused=True)
        self._zero_dev = None

    def run(self, in_maps):
        jax = self.jax
        concat = [np.concatenate([np.asarray(in_maps[c][nm])
                                  for c in range(self.n_cores)], axis=0)
                  for nm in self.in_names]
        if self._zero_dev is None:
            self._zero_dev = [
                jax.device_put(np.zeros((self.n_cores * z.shape[0], *z.shape[1:]),
                                        z.dtype))
                for z in self.zero_outs]
        args = [jax.device_put(a) for a in concat] + self._zero_dev
        outs = self.fn(*args)
        jax.block_until_ready(outs)
        return outs

    def results(self, outs):
        res = []
        for c in range(self.n_cores):
            d = {}
            for i, nm in enumerate(self.out_names):
                d[nm] = np.asarray(outs[i]).reshape(
                    self.n_cores, *self.out_avals[i].shape)[c]
            res.append(d)
        return res


def _get_compiled(stage="full"):
    if stage not in _COMPILED:
        nc = build_program(stage)
        _COMPILED[stage] = _CompiledSpmd(nc, 8)
    return _COMPILED[stage]


def make_in_maps(**inputs):
    """Build the 8 per-core input dicts from full inputs."""
    inp = {k: np.asarray(v, np.float32) for k, v in inputs.items()}
    x = np.concatenate([inp["context"] + inp["seg_context"],
                        inp["query"] + inp["seg_query"]], axis=1)  # [2, T, 512]
    W_in, conv_w, conv_b = inp["W_in"], inp["conv_w"], inp["conv_b"]
    W_x, W_dt, b_dt = inp["W_x"], inp["W_dt"], inp["b_dt"]
    negA = np.exp(inp["A_log"])  # = -A; dA = exp(A*dt) = exp(negA * ln s)
    D, W_out = inp["D"], inp["W_out"]
    Win_x, Win_z = W_in[:DF], W_in[DF:]

    in_maps = []
    metas = []
    for core in range(8):
        dirn, b, half = core // 4, (core // 2) % 2, core % 2
        xb = x[b] if dirn == 0 else x[b, ::-1]
        sl = slice(half * DH, (half + 1) * DH)
        # reorder d_inner so this core's half occupies channel blocks 0..3:
        # perm lists full-d channels with the half first.
        idx_half = np.arange(half * DH, (half + 1) * DH)
        idx_oth = np.arange((1 - half) * DH, (2 - half) * DH)
        perm = np.concatenate([idx_half, idx_oth])
        m = {
            "xT": np.ascontiguousarray(xb.T),
            "Win_l": np.ascontiguousarray(
                np.concatenate([Win_x.T[:, perm], Win_z.T[:, sl]], 1)),
            "convw": np.ascontiguousarray(conv_w[perm]),
            "convb": np.ascontiguousarray(conv_b[perm, None]),
            "Wx_l": np.ascontiguousarray(W_x.T[perm]),
            "Wdt_l": np.ascontiguousarray(
                np.concatenate([W_dt[sl].T, b_dt[None, sl]], 0)),
            "A_h": np.ascontiguousarray(negA[sl]),
            "D_h": np.ascontiguousarray(D[sl, None]),
            "Wout_l": np.ascontiguousarray(W_out[:, sl].T),
            "sel": _SEL,
            "ones_d": _ONES,
        }
        in_maps.append(m)
        metas.append((dirn, b, half))
    return in_maps, metas


def kernel(**inputs):
    Lc = np.asarray(inputs["context"]).shape[1]
    in_maps, metas = make_in_maps(**inputs)
    k = _get_compiled("full")
    outs = k.run(in_maps)
    res = k.results(outs)
    out = np.zeros((2, T - Lc, DM), np.float32)
    acc = {}
    for core, (dirn, b, half) in enumerate(metas):
        acc.setdefault((dirn, b), np.zeros((DM, T), np.float32))
        acc[(dirn, b)] += res[core]["outT"]
    for b in range(2):
        yf = acc[(0, b)].T
        yb = acc[(1, b)].T[::-1]
        out[b] = (0.5 * (yf + yb))[Lc:]
    return out.astype(np.float32)
